# revision 11
# baseline (speedup 1.0000x reference)
"""Diversity7 loss kernel for Trainium2 (8 NeuronCores, Bass/Tile).

Math (per batch row b):
  p_m   = softmax(x_m / T)                          m = 0..6, C = 1000 classes
  v_m   = (p_m - mean(p_m)) / ||p_m - mean(p_m)||   (mean(p_m) = 1/C exactly)
  q_b   = || sum_m v_m ||^2
  loss  = SCALE * mean_b((q_b - M) / 2)

Device-side restructuring (all f32):
  e   = exp(x/T)                   (ACT pass, accum_out gives Se = sum e)
  Se2 = sum e^2                    (fused affine_mul_reduce on DVE)
  Spp = Se2/Se^2;  r2 = Spp - 1/C; inv_r = exp(-0.5*ln(r2))   [per-row scalars]
  g   = inv_r/Se;  h = -inv_r/C
  s   = sum_m (g_m*e_m + h_m)     (g*e + h == v_m, centered and normalized)
  q   = sum_c s^2                  (fused affine_mul_reduce)
Host finishes in f64: loss = SCALE * mean((q-7)/2).

Sharding: data-parallel over batch. 8 cores x 512 rows; each core sees
[512,1000] slices of the 7 logit tensors and emits q for its rows as [128,4]
(partition p, row-tile rt) -> global row = core*512 + rt*128 + p.
`targets` is accepted and ignored (unused by the reference loss).
"""

import sys

import numpy as np

if "/opt/trn_rl_repo" not in sys.path:
    sys.path.insert(0, "/opt/trn_rl_repo")

import concourse.bass as bass
import concourse.tile as tile
from concourse import bacc, mybir
from concourse.bass_utils import run_bass_kernel_spmd

T = 20.0
SCALE = 0.3
C = 1000
M = 7
N_CORES = 8
ROWS_PER_CORE = 512
RT = ROWS_PER_CORE // 128  # row-tiles per core

F32 = mybir.dt.float32
AF = mybir.ActivationFunctionType
ALU = mybir.AluOpType


def _build_program() -> bass.Bass:
    nc = bacc.Bacc()
    xs = [
        nc.declare_dram_parameter(f"x{m}", [ROWS_PER_CORE, C], F32, isOutput=False)
        for m in range(M)
    ]
    q_out = nc.declare_dram_parameter("q_out", [128, RT], F32, isOutput=True)

    MT = M * RT  # 28 (model, row-tile) pairs
    with tile.TileContext(nc) as tc:
        with (
            tc.tile_pool(name="xp", bufs=3) as xp,
            tc.tile_pool(name="ep", bufs=1) as ep,
            tc.tile_pool(name="sp", bufs=2) as sp,
            tc.tile_pool(name="trp", bufs=2) as trp,
            tc.tile_pool(name="smp", bufs=1) as smp,
            tc.tile_pool(name="qp", bufs=1) as qp,
        ):
            q = qp.tile([128, RT], F32)
            Se = smp.tile([128, MT], F32, tag="Se")
            # dev2[k] = sum_c (e - Se/C)*e == Se2 - Se^2/C without cancellation
            dev2 = smp.tile([128, MT], F32, tag="dev2")

            # Phase 1: all exps first (they stay on the high-precision
            # exp table; the Ln below switches the ACT table set once).
            es: dict[tuple[int, int], bass.AP] = {}
            for rt in range(RT):
                for m in range(M):
                    k = rt * M + m
                    x = xp.tile([128, C], F32, tag="x")
                    nc.sync.dma_start(x[:], xs[m][rt * 128 : (rt + 1) * 128, :])
                    e = ep.tile([128, C], F32, tag=f"e{k}")
                    nc.scalar.activation(
                        e[:], x[:], AF.Exp, bias=0.0, scale=1.0 / T,
                        accum_out=Se[:, k : k + 1],
                    )
                    # accum = sum_c (-C*e + Se)*e  ==  -C*(Se2 - Se^2/C).
                    # scale=-C and bias=Se are exact in f32 (a rounded 1/C
                    # here would bias r2 by ~2e-5 via the x400 cancellation
                    # amplification); the 1/C lands as a harmless
                    # multiplicative rounding in the scalar math below.
                    trash = trp.tile([128, C], F32, tag="trash")
                    nc.vector.affine_mul_reduce(
                        out=trash[:], accum_out=dev2[:, k : k + 1],
                        in0=e[:], in1=e[:], scale=-float(C),
                        bias=Se[:, k : k + 1],
                    )
                    es[(rt, m)] = e

            # Phase 2: per-row scalars, batched over all 28 columns.
            invSe = smp.tile([128, MT], F32, tag="invSe")
            nc.vector.reciprocal(invSe[:], Se[:])
            # r2 = dev2 / (-C * Se^2)  (= sum_c (p - 1/C)^2, well conditioned)
            t0 = smp.tile([128, MT], F32, tag="t0")
            nc.vector.tensor_tensor(t0[:], dev2[:], invSe[:], ALU.mult)
            t1 = smp.tile([128, MT], F32, tag="t1")
            nc.vector.tensor_tensor(t1[:], t0[:], invSe[:], ALU.mult)
            r2 = smp.tile([128, MT], F32, tag="r2")
            nc.vector.tensor_scalar_mul(r2[:], t1[:], -1.0 / C)
            # rsqrt seed via ln/exp (both in the natural_log_exp set),
            # then one Newton step: y1 = y0*(1.5 - 0.5*r2*y0^2).
            lnr = smp.tile([128, MT], F32, tag="lnr")
            nc.scalar.activation(lnr[:], r2[:], AF.Ln)
            invr0 = smp.tile([128, MT], F32, tag="invr0")
            nc.scalar.activation(invr0[:], lnr[:], AF.Exp, bias=0.0, scale=-0.5)
            y0sq = smp.tile([128, MT], F32, tag="y0sq")
            nc.vector.tensor_tensor(y0sq[:], invr0[:], invr0[:], ALU.mult)
            zy = smp.tile([128, MT], F32, tag="zy")
            nc.vector.tensor_tensor(zy[:], r2[:], y0sq[:], ALU.mult)
            nrc = smp.tile([128, MT], F32, tag="nrc")
            nc.vector.tensor_scalar(
                nrc[:], zy[:], -0.5, 1.5, op0=ALU.mult, op1=ALU.add
            )
            invr = smp.tile([128, MT], F32, tag="invr")
            nc.vector.tensor_tensor(invr[:], invr0[:], nrc[:], ALU.mult)
            g = smp.tile([128, MT], F32, tag="g")
            nc.vector.tensor_tensor(g[:], invr[:], invSe[:], ALU.mult)
            h = smp.tile([128, MT], F32, tag="h")
            nc.vector.tensor_scalar_mul(h[:], invr[:], -1.0 / C)

            # Phase 3: s = sum_m (g_m*e_m + h_m);  g*e + h == v_m (centered)
            for rt in range(RT):
                s_prev = None
                for m in range(M):
                    k = rt * M + m
                    s_new = sp.tile([128, C], F32, tag="s")
                    if m == 0:
                        nc.vector.tensor_scalar(
                            s_new[:], es[(rt, 0)][:], g[:, k : k + 1],
                            h[:, k : k + 1], op0=ALU.mult, op1=ALU.add,
                        )
                    else:
                        nc.vector.affine_then_add(
                            s_new[:], es[(rt, m)][:], s_prev[:],
                            g[:, k : k + 1], h[:, k : k + 1],
                        )
                    s_prev = s_new
                trash2 = trp.tile([128, C], F32, tag="trash")
                nc.vector.affine_mul_reduce(
                    out=trash2[:], accum_out=q[:, rt : rt + 1],
                    in0=s_prev[:], in1=s_prev[:], scale=1.0, bias=0.0,
                )
            nc.sync.dma_start(q_out[:], q[:])
    return nc


_NC_CACHE: bass.Bass | None = None


def _get_program() -> bass.Bass:
    global _NC_CACHE
    if _NC_CACHE is None:
        nc = _build_program()
        nc.finalize()
        _NC_CACHE = nc
    return _NC_CACHE


def run_device_part(inputs: dict[str, np.ndarray], **run_kwargs):
    """Run the bass kernel; returns (q_all [4096] f64 row-major, BassKernelResults)."""
    nc = _get_program()
    core_ids = list(range(N_CORES))
    in_maps = []
    for c in range(N_CORES):
        lo, hi = c * ROWS_PER_CORE, (c + 1) * ROWS_PER_CORE
        in_maps.append(
            {
                f"x{m}": np.ascontiguousarray(
                    inputs[f"outputs{m + 1}"][lo:hi], dtype=np.float32
                )
                for m in range(M)
            }
        )
    res = run_bass_kernel_spmd(nc, in_maps, core_ids, **run_kwargs)
    qs = []
    for c in range(N_CORES):
        qc = np.asarray(res.results[c]["q_out"])  # [128, RT]
        qs.append(qc.T.reshape(-1))  # row = rt*128 + p order
    q_all = np.concatenate(qs).astype(np.float64)  # row = c*512 + rt*128 + p
    return q_all, res


def kernel(**inputs: np.ndarray) -> np.ndarray:
    q_all, _ = run_device_part(inputs)
    loss = SCALE * np.mean((q_all - float(M)) / 2.0)
    return np.float32(loss)
